# revision 93
# baseline (speedup 1.0000x reference)
"""GATv2 (2-layer, PyG-style) on 8 Trainium2 NeuronCores via Bass/Tile.

Self-contained: takes full inputs, shards internally (dst-node ranges x 8
cores, edge parallelism within core), returns full output.

Strategy per layer (v2 — single gather per edge):
  - phase0 (dense): xl = x @ Wl, xr = x @ Wr  -> bf16 row tables in DRAM.
  - edge phase: per chunk of <=CHUNK edges, per 128-edge tile t:
      g  = gather(xl, src)                    [e, hc]  (the ONLY per-edge DMA)
      oh[e,slot]  = (iota_row == dstrel_e)    (DVE tensor_scalar)
      ohT[slot,e] = (iota_p == dstrel_row)    (Pool pbcast + DVE tensor_scalar)
      sT_ps[hc',e] = xr_win.T @ ohT + g.T     (PE: 4+4 mms of 128 cols; the
                                               g.T term is matmul(g, I))
      vT = Prelu_0.2(sT_ps)                   (ACT, psum->sbuf bf16)
      lg_ps[e,h] += vT_j.T @ att_j            (PE: 4 mms of 8 cols)
    per chunk:
      p = exp(lg_ps)                          (ACT, [e, nsub*8])
      msgs = g * p_bcast                      (DVE TT, head-broadcast)
    per tile:
      win_ps += oh.T @ msgs ; den_ps += oh.T @ p   (PE)
  - window close: out = win/den (+bias, activation, ...) as before.

Nodes of each core are dealt degree-serpentine into 128-slot windows so the
per-window edge counts are near-equal; every (core, window) pads its edge
list to one uniform count so a single NEFF runs SPMD on all 8 cores.
"""

import math
from contextlib import ExitStack

import numpy as np
import ml_dtypes

import concourse.bass as bass
import concourse.bacc as bacc
import concourse.mybir as mybir
import concourse.tile as tile
from concourse.bass_utils import run_bass_kernel_spmd
from concourse.masks import make_identity
from concourse import library_config

P = 128
N_CORES = 8
D_IN = 64
H = 8
C_HEAD = 64
HC = H * C_HEAD            # 512
WIN = 128                  # dst slots per window
CHUNK = 768                # max edges per gather chunk
BF = mybir.dt.bfloat16
F32 = mybir.dt.float32
I16 = mybir.dt.int16

AF = mybir.ActivationFunctionType
OP = mybir.AluOpType


# ----------------------------------------------------------------------------
# host-side planning
# ----------------------------------------------------------------------------

def _wrap16(a):
    """int16 [L] -> [128, L/16] with idx j at [j%16, j//16], replicated x8."""
    L = len(a)
    assert L % 16 == 0
    w = np.asarray(a, np.int16).reshape(L // 16, 16).T  # [16, L/16]
    return np.tile(w, (8, 1))


def _wrap128(a, dtype):
    L = len(a)
    assert L % 128 == 0
    return np.ascontiguousarray(np.asarray(a, dtype).reshape(L // 128, 128).T)


def _plan(edge_index, N):
    """Bucket edges (with self-loops) by core/window; build packed index arrays.

    Returns dict with the shared static plan + per-core packed arrays.
    """
    E = edge_index.shape[1]
    src = np.concatenate([edge_index[0], np.arange(N, dtype=np.int64)])
    dst = np.concatenate([edge_index[1], np.arange(N, dtype=np.int64)])

    assert N % N_CORES == 0
    ncore = N // N_CORES
    n_win = math.ceil(ncore / WIN)
    nslots = n_win * WIN

    core_of = dst // ncore
    deg = np.bincount(dst, minlength=N)

    perms = []        # per core: slot -> node (global id), -1 if empty
    slot_of = np.full(N, -1, np.int64)   # node -> slot (within its core)
    for k in range(N_CORES):
        nodes = np.arange(k * ncore, (k + 1) * ncore)
        order = nodes[np.argsort(-deg[nodes], kind="stable")]
        perm = np.full(nslots, -1, np.int64)
        wcount = np.zeros(n_win, np.int64)
        # serpentine deal into windows
        for i, nd in enumerate(order):
            r, j = divmod(i, n_win)
            w = j if (r % 2 == 0) else (n_win - 1 - j)
            perm[w * WIN + wcount[w]] = nd
            slot_of[nd] = w * WIN + wcount[w]
            wcount[w] += 1
        assert wcount.max() <= WIN
        perms.append(perm)

    # bucket edges per (core, window)
    dslot = slot_of[dst]
    dwin = dslot // WIN
    buckets = [[[] for _ in range(n_win)] for _ in range(N_CORES)]
    order = np.argsort(core_of * n_win + dwin, kind="stable")
    for e in order:
        buckets[core_of[e]][dwin[e]].append(e)

    wmax = max(len(b) for cb in buckets for b in cb)
    wpad = ((wmax + 127) // 128) * 128
    L = n_win * wpad

    # chunk split of wpad
    chunks = []
    rem = wpad
    while rem > 0:
        c = min(CHUNK, rem)
        chunks.append(c)
        rem -= c

    cores = []
    src_gs = []
    for k in range(N_CORES):
        src_g = np.zeros(L, np.int64)
        dst_rel = np.full(L, 200.0, np.float64)
        for w in range(n_win):
            # sorted by src so early chunks only touch a prefix of the xl
            # table (lets layer-1 gathers start before phase0 finishes)
            es = sorted(buckets[k][w], key=lambda e: src[e])
            o = w * wpad
            n = len(es)
            src_g[o:o + n] = src[es]
            dst_rel[o:o + n] = dslot[es] - w * WIN
        src_gs.append(src_g)
        cores.append(dict(
            src_w=_wrap16(src_g),
            dstrel_w=_wrap128(dst_rel, np.float32),
            dstrel_row=np.asarray(dst_rel, np.float32).astype(
                ml_dtypes.bfloat16).reshape(1, L),
            perm=perms[k],
        ))

    # per-(window, chunk) max src row over all cores (+1), for sliced gathers
    rmax = []
    for w in range(n_win):
        rw = []
        off = 0
        for csz in chunks:
            o = w * wpad + off
            m = max(int(sg[o:o + csz].max()) for sg in src_gs) + 1
            rw.append(min(N, ((m + 127) // 128) * 128))
            off += csz
        rmax.append(rw)

    return dict(N=N, E=E, ncore=ncore, n_win=n_win, nslots=nslots,
                wpad=wpad, L=L, chunks=chunks, cores=cores, rmax=rmax)


# c-major hc layout: column hc' = c*H + h of every table holds (head h, chan c).
# This makes the per-head broadcast in the msgs TT contiguous on its last dim
# (stride-1 over h), which qualifies for the DVE 2x perf mode.
NEW2OLD = np.array([(g % H) * C_HEAD + g // H for g in range(HC)])


def _att_tiles(att):
    """Build [128, 4, 8] bf16 block-diagonal att weights for the logits mms.

    att_j[p, h] = att[h, c] where (j*128+p) = c*H + h (c-major layout).
    """
    out = np.zeros((P, 4, H), np.float32)
    a = np.asarray(att, np.float32)
    for j in range(4):
        for p in range(P):
            g = j * P + p
            h = g % H
            c = g // H
            out[p, j, h] = a[h, c]
    return out.astype(ml_dtypes.bfloat16)


# ----------------------------------------------------------------------------
# device kernel builders
# ----------------------------------------------------------------------------

def _edge_phase(nc, tc, ctx, plan, xl_tbl, xr_sb, sb_idx, sb_const, close_fn,
                den_bufs=1, s_bufs=3, s_group=1, prelu_dve_mod=0,
                use_rmax=False, xr_dram=None, late_loads=()):
    """Shared edge-phase + per-window close loop (v2 layout).

    xl_tbl: DRAM AP of bf16 row table ([*, 512]); gathered by src.
    xr_sb: resident SBUF tile AP [P, n_win, 512] (slot-major xr table).
    use_rmax: slice xl_tbl to plan['rmax'][w][ci] rows per gather so reads
              only depend on the written prefix (layer 1 only).
    sb_idx: dict with src_w ([P, L/16] i16), dstrel_w ([P, L/128] f32) SBUF
            tiles and dstrow ([1, L] bf16) DRAM AP (window slices are DMAed
            into a small pool here).
    sb_const: dict with att_s ([128,4,8] bf16), iota ([128,128] bf16),
              iota_p ([128,1] f32), identb ([128,128] bf16).
    close_fn(w, win_ps, den_ps): consume the accumulated window psum.
    """
    n_win, wpad, chunks = plan["n_win"], plan["wpad"], plan["chunks"]

    gp = ctx.enter_context(tc.tile_pool(name="gather", bufs=5))
    ep = ctx.enter_context(tc.tile_pool(name="edges", bufs=6))
    ohp = ctx.enter_context(tc.tile_pool(name="onehot", bufs=6))
    wpp = ctx.enter_context(tc.tile_pool(name="winper", bufs=2))
    vp = ctx.enter_context(tc.tile_pool(name="vt", bufs=12))
    pp = ctx.enter_context(tc.tile_pool(name="small", bufs=6))
    ps_s = ctx.enter_context(tc.tile_pool(name="psst", bufs=s_bufs, space="PSUM"))
    ps_lg = ctx.enter_context(tc.tile_pool(name="pslg", bufs=1, space="PSUM"))
    ps_win = ctx.enter_context(tc.tile_pool(name="pswin", bufs=2, space="PSUM"))
    ps_den = ctx.enter_context(
        tc.tile_pool(name="psden", bufs=den_bufs, space="PSUM"))

    att_s = sb_const["att_s"]
    iota, iota_p = sb_const["iota"], sb_const["iota_p"]
    identb = sb_const["identb"]

    nsub_total = wpad // 128
    # deferred work: chunk aggregations and window closes are emitted one
    # step later so their wait-bound PE/DVE instructions don't head-of-line
    # block the next chunk's (or window's) independent work in the queues
    pending = []          # deque of (wstate, oh, msgs, p, nsub)
    pending_close = None  # wstate
    PEND_DEPTH = 4

    def emit_agg(pend):
        ws, oh_, msgs_, p_, nsub_ = pend
        for t in range(nsub_):
            first = (ws["done"] == 0)
            last = (ws["done"] == nsub_total - 1)
            nc.tensor.matmul(ws["den"][:], oh_[:, t, :], p_[:, t, :],
                             start=first, stop=last)
            nc.tensor.matmul(ws["win"][:], oh_[:, t, :], msgs_[:, t, :],
                             start=first, stop=last)
            ws["done"] += 1

    for w in range(n_win):
        win_ps = ps_win.tile([P, HC], F32, tag="win")
        den_ps = ps_den.tile([P, H], F32, tag="den")
        wstate = dict(w=w, win=win_ps, den=den_ps, done=0)
        if xr_dram is not None:
            nc.sync.dma_start(xr_sb[:, w, :],
                              xr_dram[w * WIN:(w + 1) * WIN, :])
        xr_win = xr_sb[:, w, :]
        drow = wpp.tile([1, wpad], BF, tag="drow")
        nc.sync.dma_start(drow[:], sb_idx["dstrow"][:, w * wpad:(w + 1) * wpad])
        # whole-window transposed one-hot: ohT[slot, e] = (slot == dstrel_e)
        drB = wpp.tile([P, wpad], BF, tag="drB")
        nc.gpsimd.partition_broadcast(drB[:], drow[:])
        ohT = wpp.tile([P, wpad], BF, tag="ohT")
        nc.vector.tensor_scalar(ohT[:], drB[:], iota_p, None, op0=OP.is_equal)
        e0 = w * wpad

        for ci, csz in enumerate(chunks):
            co = e0 + sum(chunks[:ci])
            nsub = csz // 128
            # --- gather xl[src] (the only per-edge DMA) ---
            g = gp.tile([P, nsub, HC], BF, tag="g")
            isl = sb_idx["src_w"][:, co // 16:(co + csz) // 16]
            tbl = (xl_tbl[0:plan["rmax"][w][ci], :] if use_rmax else xl_tbl)
            nc.gpsimd.dma_gather(g[:], tbl, isl, csz, csz, HC)
            if w == 0 and ci == 0:
                for dst_ap, src_ap in late_loads:
                    nc.sync.dma_start(dst_ap, src_ap)

            # --- one-hots (edge-major orientation) ---
            oh = ohp.tile([P, nsub, P], BF, tag="oh")
            lg_ps = ps_lg.tile([P, nsub, H], F32, tag="lg")
            p = pp.tile([P, nsub, H], BF, tag="p")
            # group s_group tiles per PSUM tile so one Prelu covers the group
            groups = []
            t0 = 0
            while t0 < nsub:
                groups.append((t0, min(s_group, nsub - t0)))
                t0 += s_group
            s_tiles = {}
            for t0, gn in groups:
                s_ps = ps_s.tile([P, s_group * 4, P], F32, tag="s",
                                 name=f"s{w}_{ci}_{t0}")
                s_tiles[t0] = s_ps
                for ti in range(gn):
                    t = t0 + ti
                    gsub = co // 128 + t
                    nc.vector.tensor_scalar(
                        oh[:, t, :], iota[:],
                        sb_idx["dstrel_w"][:, gsub:gsub + 1], None,
                        op0=OP.is_equal)
                    esub = sum(chunks[:ci]) + t * 128
                    # --- sT = g.T + xr[dst].T  (PSUM [hc', e]) ---
                    for j in range(4):
                        nc.tensor.matmul(
                            s_ps[:, ti * 4 + j, :], g[:, t, j * P:(j + 1) * P],
                            identb[:], start=True, stop=False)
                        nc.tensor.matmul(
                            s_ps[:, ti * 4 + j, :], xr_win[:, j * P:(j + 1) * P],
                            ohT[:, esub:esub + P], start=False, stop=True)
            if len(pending) >= PEND_DEPTH:
                pend = pending.pop(0)
                emit_agg(pend)
                pws = pend[0]
                if pws["done"] == nsub_total:
                    pending_close = pws
            if pending_close is not None:
                close_fn(pending_close["w"], pending_close["win"],
                         pending_close["den"])
                pending_close = None
            for gi, (t0, gn) in enumerate(groups):
                s_ps = s_tiles[t0]
                # --- vT = lrelu(sT) ---
                vT = vp.tile([P, gn * 4, P], BF, tag=f"vt{gn}",
                             name=f"vt{w}_{ci}_{t0}")
                if (prelu_dve_mod and gn == s_group
                        and (w * 7 + ci * 3 + gi) % prelu_dve_mod == 0):
                    s2 = vp.tile([P, gn * 4, P], BF, tag=f"s2{gn}",
                                 name=f"s2_{w}_{ci}_{t0}")
                    nc.vector.tensor_scalar_mul(s2[:], s_ps[:, 0:gn * 4, :],
                                                0.2)
                    nc.vector.tensor_tensor(vT[:], s_ps[:, 0:gn * 4, :],
                                            s2[:], op=OP.max)
                else:
                    nc.scalar.activation(vT[:], s_ps[:, 0:gn * 4, :], AF.Prelu,
                                         alpha=0.2)
                # --- logits[e, h] += vT_j.T @ att_j ---
                for ti in range(gn):
                    t = t0 + ti
                    for j in range(4):
                        nc.tensor.matmul(lg_ps[:, t, :], vT[:, ti * 4 + j, :],
                                         att_s[:, j, :],
                                         start=(j == 0), stop=(j == 3))

            # --- p = exp(logits), msgs = g * p (head-broadcast) ---
            nc.scalar.activation(p[:], lg_ps[:], AF.Exp)
            msgs = ep.tile([P, nsub, HC], BF, tag="msgs")
            nc.vector.tensor_tensor(
                msgs[:].rearrange("p s (c h) -> p s c h", h=H),
                g[:].rearrange("p s (c h) -> p s c h", h=H),
                p[:].rearrange("p s (c h) -> p s c h", c=1).to_broadcast(
                    [P, nsub, C_HEAD, H]), op=OP.mult)

            pending.append((wstate, oh, msgs, p, nsub))

    # drain the deferred tail
    for pend in pending:
        emit_agg(pend)
        pws = pend[0]
        if pws["done"] == nsub_total:
            if pending_close is not None:
                close_fn(pending_close["w"], pending_close["win"],
                         pending_close["den"])
            pending_close = pws
    if pending_close is not None:
        close_fn(pending_close["w"], pending_close["win"],
                 pending_close["den"])


def build_neff_a(plan):
    """Layer 1 NEFF: x -> h -> xl2_loc/xr2_loc (slot order, bf16)."""
    N, nslots, n_win, L = plan["N"], plan["nslots"], plan["n_win"], plan["L"]
    nm = N // P + (1 if N % P else 0)

    nc = bacc.Bacc("TRN2", target_bir_lowering=False, debug=False,
                   num_devices=N_CORES)
    xT = nc.dram_tensor("xT", [D_IN, N], BF, kind="ExternalInput")
    xTp = nc.dram_tensor("xTp", [D_IN, nslots], BF, kind="ExternalInput")
    wl1 = nc.dram_tensor("wl1", [D_IN, HC], BF, kind="ExternalInput")
    wr1 = nc.dram_tensor("wr1", [D_IN, HC], BF, kind="ExternalInput")
    wl2 = nc.dram_tensor("wl2", [HC, HC], BF, kind="ExternalInput")
    wr2 = nc.dram_tensor("wr2", [HC, HC], BF, kind="ExternalInput")
    atts = nc.dram_tensor("atts", [P, 4 * H], BF, kind="ExternalInput")
    b1r = nc.dram_tensor("b1r", [1, HC], F32, kind="ExternalInput")
    srcw = nc.dram_tensor("srcw", [P, L // 16], I16, kind="ExternalInput")
    dstrw = nc.dram_tensor("dstrw", [P, L // 128], F32, kind="ExternalInput")
    dstrow = nc.dram_tensor("dstrow", [1, L], BF, kind="ExternalInput")
    xl2o = nc.dram_tensor("xl2o", [nslots, HC], BF, kind="ExternalOutput")
    xr2o = nc.dram_tensor("xr2o", [nslots, HC], BF, kind="ExternalOutput")

    with tile.TileContext(nc) as tc, ExitStack() as ctx:
        nc.gpsimd.load_library(library_config.mlp)
        res = ctx.enter_context(tc.tile_pool(name="res", bufs=1))
        dram = ctx.enter_context(tc.tile_pool(name="dram", bufs=1, space="DRAM"))

        # resident constants / tables
        wl2_sb = res.tile([P, 4, HC], BF)
        nc.sync.dma_start(wl2_sb[:], wl2[:].rearrange("(k p) n -> p k n", p=P))
        wr2_sb = res.tile([P, 4, HC], BF)
        nc.sync.dma_start(wr2_sb[:], wr2[:].rearrange("(k p) n -> p k n", p=P))
        att_s = res.tile([P, 4, H], BF)
        nc.sync.dma_start(att_s[:], atts[:].rearrange("p (j h) -> p j h", h=H))
        b1b = res.tile([P, HC], F32)
        nc.sync.dma_start(b1b[:], b1r[:].to_broadcast([P, HC]))
        wp16 = plan["wpad"] // 16
        wp128 = plan["wpad"] // 128
        src_w = res.tile([P, L // 16], I16)
        nc.sync.dma_start(src_w[:, 0:wp16], srcw[:, 0:wp16])
        dstrel_w = res.tile([P, L // 128], F32)
        nc.sync.dma_start(dstrel_w[:, 0:wp128], dstrw[:, 0:wp128])
        late = [(src_w[:, wp16:], srcw[:, wp16:]),
                (dstrel_w[:, wp128:], dstrw[:, wp128:])]
        iota_i = res.tile([P, P], I16)
        nc.gpsimd.iota(iota_i[:], pattern=[[1, P]], base=0, channel_multiplier=0)
        iota = res.tile([P, P], BF)
        nc.vector.tensor_copy(iota[:], iota_i[:])
        iotap_i = res.tile([P, 1], I16)
        nc.gpsimd.iota(iotap_i[:], pattern=[[0, 1]], base=0,
                       channel_multiplier=1)
        iota_p = res.tile([P, 1], F32)
        nc.vector.tensor_copy(iota_p[:], iotap_i[:])
        identb = res.tile([P, P], BF)
        make_identity(nc, identb[:])
        hT_tiles = [res.tile([P, 4, WIN], BF, tag=f"ht{w}", name=f"ht{w}")
                    for w in range(n_win)]
        xr_sb = res.tile([P, n_win, HC], BF, name="xr_sb")

        xl1 = dram.tile([N, HC], BF)

        # ---- phase 0: xl1 = x @ Wl1 (all rows), xr1 = x_perm @ Wr1 ----
        with tc.tile_pool(name="p0ps", bufs=4, space="PSUM") as p0ps, \
             tc.tile_pool(name="p0sb", bufs=10) as p0sb, \
             tc.tile_pool(name="p0big", bufs=1) as p0big:
            xT_sb = p0big.tile([D_IN, N], BF)
            wl1_sb = p0big.tile([D_IN, HC], BF)
            wr1_sb = p0big.tile([D_IN, HC], BF)
            xTp_sb = p0big.tile([D_IN, nslots], BF)
            # dependency-ordered loads: first mm needs wl1 + xT prefix only.
            # Bulk loads ride the Activation HWDGE queue so the table writes
            # on the SP queue are not stuck behind them.
            nc.sync.dma_start(wl1_sb[:], wl1[:])
            nc.sync.dma_start(xT_sb[:, 0:2560], xT[:, 0:2560])
            nc.sync.dma_start(wr1_sb[:], wr1[:])
            nc.sync.dma_start(xT_sb[:, 2560:], xT[:, 2560:])
            nc.sync.dma_start(xTp_sb[:], xTp[:])
            def p0_pairs(n_tiles, n_rows, lhs_sb, w_sb, out_tbl, tag):
                for m2 in range(0, n_tiles, 2):
                    nt = min(2, n_tiles - m2)
                    xps = p0ps.tile([P, nt, HC], F32, tag="p0",
                                    name=f"{tag}ps{m2}")
                    rws = []
                    for i in range(nt):
                        m = m2 + i
                        rows = min(P, n_rows - m * P)
                        rws.append(rows)
                        nc.tensor.matmul(
                            xps[:rows, i, :], lhs_sb[:, m * P:m * P + rows],
                            w_sb[:], start=True, stop=True)
                    if out_tbl is None:
                        # xr table: straight into the resident SBUF tile
                        if (m2 // 2) % 2 == 0:
                            nc.vector.tensor_copy(xr_sb[:, m2:m2 + nt, :],
                                                  xps[:])
                        else:
                            nc.scalar.copy(xr_sb[:, m2:m2 + nt, :], xps[:])
                        continue
                    xsb = p0sb.tile([P, nt, HC], BF, tag=tag + "sb",
                                    name=f"{tag}sb{m2}")
                    if (m2 // 2) % 2 == 0:
                        nc.vector.tensor_copy(xsb[:], xps[:])
                    else:
                        nc.scalar.copy(xsb[:], xps[:])
                    if nt == 2 and rws[0] == P and rws[1] == P:
                        nc.sync.dma_start(
                            out_tbl[m2 * P:(m2 + 2) * P, :].rearrange(
                                "(b p) c -> p b c", b=2), xsb[:])
                    else:
                        for i in range(nt):
                            m = m2 + i
                            nc.sync.dma_start(
                                out_tbl[m * P:m * P + rws[i], :],
                                xsb[:rws[i], i, :])

            p0_pairs(nm, N, xT_sb[:], wl1_sb[:], xl1, "xl")
            p0_pairs(n_win, nslots, xTp_sb[:], wr1_sb[:], None, "xr")

        # ---- edge phase + close: h = elu(win/den + b1), hT resident ----
        sb_idx = dict(src_w=src_w[:], dstrel_w=dstrel_w[:],
                      dstrow=dstrow[:])
        sb_const = dict(att_s=att_s[:], iota=iota[:], iota_p=iota_p[:],
                        identb=identb[:])

        with ExitStack() as ectx:
            cl = ectx.enter_context(tc.tile_pool(name="close", bufs=2))
            ps_cl = ectx.enter_context(
                tc.tile_pool(name="pscl", bufs=1, space="PSUM"))

            def close1(w, win_ps, den_ps):
                den_r = cl.tile([P, H], F32, tag="denr")
                nc.vector.reciprocal(den_r[:], den_ps[:])
                x = cl.tile([P, HC], F32, tag="x")
                nc.vector.tensor_tensor(
                    x[:].rearrange("p (c h) -> p c h", h=H),
                    win_ps[:].rearrange("p (c h) -> p c h", h=H),
                    den_r[:].rearrange("p (c h) -> p c h", c=1).to_broadcast(
                        [P, C_HEAD, H]), op=OP.mult)
                nc.vector.tensor_tensor(x[:], x[:], b1b[:], op=OP.add)
                xm = cl.tile([P, HC], F32, tag="xm")
                nc.vector.tensor_scalar(xm[:], x[:], 0.0, None, op0=OP.min)
                ex = cl.tile([P, HC], F32, tag="ex")
                nc.scalar.activation(ex[:], xm[:], AF.Exp)
                xp = cl.tile([P, HC], F32, tag="xp")
                nc.vector.tensor_scalar(xp[:], x[:], 0.0, None, op0=OP.max)
                hbf = cl.tile([P, HC], BF, tag="hbf")
                nc.vector.scalar_tensor_tensor(hbf[:], ex[:], -1.0, xp[:],
                                               op0=OP.add, op1=OP.add)
                htp = ps_cl.tile([P, 4, P], BF, tag="cl", name=f"htp{w}")
                for j in range(4):
                    nc.tensor.transpose(htp[:, j, :],
                                        hbf[:, j * P:(j + 1) * P], identb[:])
                nc.scalar.copy(hT_tiles[w][:], htp[:])
                # fused phase0b: xl2/xr2 rows for this window (overlaps edge
                # phase of later windows)
                msl = slice(w * P, (w + 1) * P)
                aps = ps_cl.tile([P, HC], F32, tag="cl", name=f"aps{w}")
                for k in range(4):
                    nc.tensor.matmul(aps[:], hT_tiles[w][:, k, :],
                                     wl2_sb[:, k, :],
                                     start=(k == 0), stop=(k == 3))
                asb = cl.tile([P, HC], BF, tag="asb")
                if w % 2 == 0:
                    nc.vector.tensor_copy(asb[:], aps[:])
                else:
                    nc.scalar.copy(asb[:], aps[:])
                nc.sync.dma_start(xl2o[msl, :], asb[:])
                bps = ps_cl.tile([P, HC], F32, tag="cl", name=f"bps{w}")
                for k in range(4):
                    nc.tensor.matmul(bps[:], hT_tiles[w][:, k, :],
                                     wr2_sb[:, k, :],
                                     start=(k == 0), stop=(k == 3))
                bsb = cl.tile([P, HC], BF, tag="bsb")
                if w % 2 == 0:
                    nc.scalar.copy(bsb[:], bps[:])
                else:
                    nc.vector.tensor_copy(bsb[:], bps[:])
                nc.sync.dma_start(xr2o[msl, :], bsb[:])

            _edge_phase(nc, tc, ectx, plan, xl1[:], xr_sb[:], sb_idx, sb_const,
                        close1, use_rmax=True, late_loads=late)

    nc.compile()
    return nc


def build_neff_b(plan):
    """Layer 2 NEFF: xl2_full/xr2_loc tables -> out rows (slot order, f32)."""
    N, nslots, n_win, L = plan["N"], plan["nslots"], plan["n_win"], plan["L"]

    nc = bacc.Bacc("TRN2", target_bir_lowering=False, debug=False,
                   num_devices=N_CORES)
    xl2 = nc.dram_tensor("xl2", [N, HC], BF, kind="ExternalInput")
    xr2 = nc.dram_tensor("xr2", [nslots, HC], BF, kind="ExternalInput")
    atts = nc.dram_tensor("atts", [P, 4 * H], BF, kind="ExternalInput")
    b2r = nc.dram_tensor("b2r", [1, C_HEAD], F32, kind="ExternalInput")
    srcw = nc.dram_tensor("srcw", [P, L // 16], I16, kind="ExternalInput")
    dstrw = nc.dram_tensor("dstrw", [P, L // 128], F32, kind="ExternalInput")
    dstrow = nc.dram_tensor("dstrow", [1, L], BF, kind="ExternalInput")
    outo = nc.dram_tensor("outo", [nslots, C_HEAD], F32, kind="ExternalOutput")

    with tile.TileContext(nc) as tc, ExitStack() as ctx:
        nc.gpsimd.load_library(library_config.mlp)
        res = ctx.enter_context(tc.tile_pool(name="res", bufs=1))

        att_s = res.tile([P, 4, H], BF)
        nc.sync.dma_start(att_s[:], atts[:].rearrange("p (j h) -> p j h", h=H))
        b2b = res.tile([P, C_HEAD], F32)
        nc.sync.dma_start(b2b[:], b2r[:].to_broadcast([P, C_HEAD]))
        src_w = res.tile([P, L // 16], I16)
        nc.sync.dma_start(src_w[:], srcw[:])
        dstrel_w = res.tile([P, L // 128], F32)
        nc.sync.dma_start(dstrel_w[:], dstrw[:])
        iota_i = res.tile([P, P], I16)
        nc.gpsimd.iota(iota_i[:], pattern=[[1, P]], base=0, channel_multiplier=0)
        iota = res.tile([P, P], BF)
        nc.vector.tensor_copy(iota[:], iota_i[:])
        iotap_i = res.tile([P, 1], I16)
        nc.gpsimd.iota(iotap_i[:], pattern=[[0, 1]], base=0,
                       channel_multiplier=1)
        iota_p = res.tile([P, 1], F32)
        nc.vector.tensor_copy(iota_p[:], iotap_i[:])
        identb = res.tile([P, P], BF)
        make_identity(nc, identb[:])
        xr_sb = res.tile([P, n_win, HC], BF, name="xr_sb")

        sb_idx = dict(src_w=src_w[:], dstrel_w=dstrel_w[:],
                      dstrow=dstrow[:])
        sb_const = dict(att_s=att_s[:], iota=iota[:], iota_p=iota_p[:],
                        identb=identb[:])

        with ExitStack() as ectx:
            cl = ectx.enter_context(tc.tile_pool(name="close", bufs=2))

            def close2(w, win_ps, den_ps):
                den_r = cl.tile([P, H], F32, tag="denr")
                nc.vector.reciprocal(den_r[:], den_ps[:])
                x = cl.tile([P, HC], F32, tag="x")
                nc.vector.tensor_tensor(
                    x[:].rearrange("p (c h) -> p c h", h=H),
                    win_ps[:].rearrange("p (c h) -> p c h", h=H),
                    den_r[:].rearrange("p (c h) -> p c h", c=1).to_broadcast(
                        [P, C_HEAD, H]), op=OP.mult)
                m8 = cl.tile([P, C_HEAD], F32, tag="m8")
                nc.vector.tensor_reduce(
                    m8[:], x[:].rearrange("p (c h) -> p c h", h=H),
                    axis=mybir.AxisListType.X, op=OP.add)
                orow = cl.tile([P, C_HEAD], F32, tag="orow")
                nc.vector.scalar_tensor_tensor(orow[:], m8[:], 1.0 / H, b2b[:],
                                               op0=OP.mult, op1=OP.add)
                nc.sync.dma_start(outo[w * WIN:(w + 1) * WIN, :], orow[:])

            _edge_phase(nc, tc, ectx, plan, xl2[:], xr_sb[:], sb_idx, sb_const,
                        close2, den_bufs=2, s_bufs=3, xr_dram=xr2[:])

    nc.compile()
    return nc


# ----------------------------------------------------------------------------
# entry point
# ----------------------------------------------------------------------------

_cache = {}

# Cost-model (TimelineSim) per-NEFF execution-time estimate of the last
# compiled pair, in ns (summed).  No NTFF profiling is available through
# this environment's axon path, so this is the best per-NEFF HW-time proxy.
LAST_LAUNCH_NS = None


def kernel(x, edge_index, Wl1, Wr1, att1, b1, Wl2, Wr2, att2, b2):
    global LAST_LAUNCH_NS
    x = np.asarray(x, np.float32)
    edge_index = np.asarray(edge_index)
    N = x.shape[0]

    plan = _plan(edge_index, N)
    key = (N, plan["wpad"])
    if key not in _cache:
        neff_a = build_neff_a(plan)
        neff_b = build_neff_b(plan)
        ns = None
        try:
            from concourse.timeline_sim import TimelineSim
            ns = (TimelineSim(neff_a, require_finite=False,
                              require_nnan=False).simulate()
                  + TimelineSim(neff_b, require_finite=False,
                                require_nnan=False).simulate())
        except Exception:
            pass
        _cache[key] = (neff_a, neff_b, ns)
    neff_a, neff_b, LAST_LAUNCH_NS = _cache[key]

    bf = ml_dtypes.bfloat16
    att1_s = _att_tiles(att1)
    att2_s = _att_tiles(att2)
    xT = np.ascontiguousarray(x.T.astype(bf))
    # c-major hc permutation of all hc-indexed table axes
    po = NEW2OLD
    wl1_b = np.asarray(Wl1, np.float32)[:, po].astype(bf)
    wr1_b = np.asarray(Wr1, np.float32)[:, po].astype(bf)
    wl2_b = np.asarray(Wl2, np.float32)[po][:, po].astype(bf)
    wr2_b = np.asarray(Wr2, np.float32)[po][:, po].astype(bf)
    b1_p = np.asarray(b1, np.float32)[po]

    in_a = []
    for k in range(N_CORES):
        ck = plan["cores"][k]
        perm = ck["perm"]
        xperm = np.zeros((plan["nslots"], D_IN), np.float32)
        valid = perm >= 0
        xperm[valid] = x[perm[valid]]
        in_a.append(dict(
            xT=xT, xTp=np.ascontiguousarray(xperm.T.astype(bf)),
            wl1=wl1_b, wr1=wr1_b, wl2=wl2_b, wr2=wr2_b,
            atts=att1_s.reshape(P, 4 * H),
            b1r=b1_p.reshape(1, HC),
            srcw=ck["src_w"], dstrw=ck["dstrel_w"], dstrow=ck["dstrel_row"],
        ))

    res_a = run_bass_kernel_spmd(neff_a, in_a, core_ids=list(range(N_CORES)))

    # host re-assembly of the global xl2 table (slot order -> natural order)
    xl2_full = np.zeros((N, HC), bf)
    for k in range(N_CORES):
        perm = plan["cores"][k]["perm"]
        valid = perm >= 0
        xl2_full[perm[valid]] = res_a.results[k]["xl2o"][valid]

    in_b = []
    for k in range(N_CORES):
        ck = plan["cores"][k]
        in_b.append(dict(
            xl2=xl2_full, xr2=res_a.results[k]["xr2o"],
            atts=att2_s.reshape(P, 4 * H),
            b2r=np.asarray(b2, np.float32).reshape(1, C_HEAD),
            srcw=ck["src_w"], dstrw=ck["dstrel_w"], dstrow=ck["dstrel_row"],
        ))

    res_b = run_bass_kernel_spmd(neff_b, in_b, core_ids=list(range(N_CORES)))

    out = np.zeros((N, C_HEAD), np.float32)
    for k in range(N_CORES):
        perm = plan["cores"][k]["perm"]
        valid = perm >= 0
        out[perm[valid]] = res_b.results[k]["outo"][valid]
    return out
